# revision 5
# baseline (speedup 1.0000x reference)
"""Trainium2 Bass kernel for the CNF log-density problem.

Computes log p(z | cond) for a continuous normalizing flow: 10 Tsit5 steps
(6 stages each) of the augmented ODE (state + log-density), batch 256,
data-parallel over 8 NeuronCores (32 samples/core).

Key algorithmic transform: the reference computes the Jacobian trace with 32
VJPs; here it is evaluated in closed form.  For the 3-layer ConcatSquash MLP

    f(z) = L2( tanh(L1( tanh(L0([z;c])) )) ),  Li(h) = (Wi h + bi)*gi(t) + si(t)

the Jacobian trace is

    tr(J) = u0^T Btil(t) u1,     u = 1 - tanh(a)^2 at layers 0/1,
    Btil_ij = dt*b_s * g0_i * [W0z diag(g2) W2]_ij * (W1)_ji * g1_j

which depends on the sample only through u0, u1.  Btil (one 128x128 matrix
per stage-time, 60 total) plus all gate scale/bias vectors are precomputed on
the host from params and the fixed time grid.

Device layout: features on partitions, samples on the free dim (32/core).
The RK stage combinations z_i = z + sum_j a_ij k_j are folded into the first
matmul's PSUM accumulation (prescaled W0z slices), keeping the serial chain
per stage at mm -> tanh -> mm -> tanh -> mm -> gate.
"""

import numpy as np

import concourse.bacc as bacc
import concourse.mybir as mybir
import concourse.tile as tile

# ---- problem constants (hardcoded per spec) ----
NUM_LATENTS, NUM_CONDS, WIDTH = 32, 32, 128
BATCH, N_CORES = 256, 8
SPC = BATCH // N_CORES  # samples per core = 32
T1, DT = 1.0, -0.1
N_STEPS = 10
N_STAGES = 6
LOG_2PI = float(np.log(2.0 * np.pi))

TSIT_C = [0.0, 0.161, 0.327, 0.9, 0.9800255409045097, 1.0]
TSIT_A = [
    [],
    [0.161],
    [-0.008480655492356989, 0.335480655492357],
    [2.8971530571054935, -6.359448489975075, 4.3622954328695815],
    [5.325864828439257, -11.748883564062828, 7.4955393428898365, -0.09249506636175525],
    [5.86145544294642, -12.92096931784711, 8.159367898576159, -0.071584973281401, -0.028269050394068383],
]
TSIT_B = [0.09646076681806523, 0.01, 0.4798896504144996, 1.379008574103742, -3.290069515436081, 2.324710524099774]

NSTG = N_STEPS * N_STAGES  # 60
# flat index for the 15 (i, j) lower-triangular A coefficients
_AIDX = {(i, j): n for n, (i, j) in enumerate((i, j) for i in range(N_STAGES) for j in range(i))}
N_APAIRS = len(_AIDX)  # 15

F32 = mybir.dt.float32


def _build_program():
    nc = bacc.Bacc("TRN2", target_bir_lowering=False, debug=False, num_devices=N_CORES)

    # ---- DRAM I/O ----
    xzc_d = nc.dram_tensor("xzc", [64, SPC], F32, kind="ExternalInput")          # [z;cond] feature-major
    w0t_d = nc.dram_tensor("w0t", [64, WIDTH], F32, kind="ExternalInput")        # W1_0^T
    w1t_d = nc.dram_tensor("w1t", [WIDTH, WIDTH], F32, kind="ExternalInput")     # W1_1^T
    w2t_d = nc.dram_tensor("w2t", [WIDTH, NUM_LATENTS], F32, kind="ExternalInput")  # W1_2^T
    wz_d = nc.dram_tensor("wz", [NUM_LATENTS, N_APAIRS * WIDTH], F32, kind="ExternalInput")  # (dt*a_ij*W0z)^T
    bt_d = nc.dram_tensor("bt", [WIDTH, NSTG * WIDTH], F32, kind="ExternalInput")   # Btil per stage
    sb128_d = nc.dram_tensor("sb128", [WIDTH, NSTG * 5 + 1], F32, kind="ExternalInput")  # sc0|bi0|sc1|bi1|c, +negones
    sb32_d = nc.dram_tensor("sb32", [NUM_LATENTS, NSTG * 2 + 2], F32, kind="ExternalInput")  # sc2|bi2, +neghalf, +final bias
    out_d = nc.dram_tensor("out", [1, SPC], F32, kind="ExternalOutput")

    with tile.TileContext(nc) as tc:
        with (
            tc.tile_pool(name="const", bufs=1) as cpool,
            tc.tile_pool(name="bt", bufs=2) as btpool,
            tc.tile_pool(name="work", bufs=3) as wpool,
            tc.tile_pool(name="kzp", bufs=2) as kzpool,
            tc.tile_pool(name="psA", bufs=2, space="PSUM") as psA,
            tc.tile_pool(name="psB", bufs=2, space="PSUM") as psB,
            tc.tile_pool(name="psF", bufs=1, space="PSUM") as psF,
            tc.tile_pool(name="psM", bufs=2, space="PSUM") as psM,
            tc.tile_pool(name="psacc", bufs=1, space="PSUM") as psacc,
        ):
            # ---- persistent constants ----
            w0t = cpool.tile([64, WIDTH], F32)
            w1t = cpool.tile([WIDTH, WIDTH], F32)
            w2t = cpool.tile([WIDTH, NUM_LATENTS], F32)
            wz = cpool.tile([NUM_LATENTS, N_APAIRS * WIDTH], F32)
            sb128 = cpool.tile([WIDTH, NSTG * 5 + 1], F32)
            sb32 = cpool.tile([NUM_LATENTS, NSTG * 2 + 2], F32)
            xzc0 = cpool.tile([64, SPC], F32, tag="xzc0")
            xzc1 = cpool.tile([64, SPC], F32, tag="xzc1")
            res = cpool.tile([1, SPC], F32, tag="res")

            nc.sync.dma_start(w0t[:], w0t_d[:])
            nc.sync.dma_start(w1t[:], w1t_d[:])
            nc.sync.dma_start(w2t[:], w2t_d[:])
            nc.sync.dma_start(wz[:], wz_d[:])
            nc.sync.dma_start(sb128[:], sb128_d[:])
            nc.sync.dma_start(sb32[:], sb32_d[:])
            nc.sync.dma_start(xzc0[:], xzc_d[:])
            # cond half is shared by both ping/pong state tiles
            nc.vector.tensor_copy(xzc1[32:64, :], xzc0[32:64, :])

            negones = sb128[:, NSTG * 5 : NSTG * 5 + 1]   # (128,1) of -1
            neghalf = sb32[:, NSTG * 2 : NSTG * 2 + 1]    # (32,1) of -0.5

            acc = psacc.tile([1, SPC], F32)

            s = 0  # flat stage index
            for n in range(N_STEPS):
                cur = xzc0 if n % 2 == 0 else xzc1
                nxt = xzc1 if n % 2 == 0 else xzc0
                # Btil chunk for this step (6 stages x 128 cols)
                btc = btpool.tile([WIDTH, N_STAGES * WIDTH], F32, tag="btc")
                nc.sync.dma_start(btc[:], bt_d[:, n * N_STAGES * WIDTH : (n + 1) * N_STAGES * WIDTH])

                # start z accumulator for this step: znext = z
                nc.vector.tensor_copy(nxt[0:32, :], cur[0:32, :])

                kz = kzpool.tile([NUM_LATENTS, N_STAGES * SPC], F32, tag="kz")

                for i in range(N_STAGES):
                    sc0 = sb128[:, 5 * s + 0 : 5 * s + 1]
                    bi0 = sb128[:, 5 * s + 1 : 5 * s + 2]
                    sc1 = sb128[:, 5 * s + 2 : 5 * s + 3]
                    bi1 = sb128[:, 5 * s + 3 : 5 * s + 4]
                    cvec = sb128[:, 5 * s + 4 : 5 * s + 5]
                    sc2 = sb32[:, 2 * s + 0 : 2 * s + 1]
                    bi2 = sb32[:, 2 * s + 1 : 2 * s + 2]

                    # ---- layer 0: a0 = W0 @ [z;c] + sum_j dt*a_ij * W0z @ k_j ----
                    a0 = psA.tile([WIDTH, SPC], F32, tag="a0")
                    nc.tensor.matmul(a0[:], w0t[:], cur[:], start=True, stop=(i == 0))
                    for j in range(i):
                        widx = _AIDX[(i, j)]
                        nc.tensor.matmul(
                            a0[:],
                            wz[:, widx * WIDTH : (widx + 1) * WIDTH],
                            kz[:, j * SPC : (j + 1) * SPC],
                            start=False,
                            stop=(j == i - 1),
                        )
                    t0 = wpool.tile([WIDTH, SPC], F32, tag="t0")
                    nc.scalar.activation(t0[:], a0[:], mybir.ActivationFunctionType.Tanh, bias=bi0, scale=sc0)
                    sq0 = wpool.tile([WIDTH, SPC], F32, tag="sq0")
                    nc.scalar.activation(sq0[:], t0[:], mybir.ActivationFunctionType.Square)

                    # ---- layer 1 ----
                    a1 = psB.tile([WIDTH, SPC], F32, tag="a1")
                    nc.tensor.matmul(a1[:], w1t[:], t0[:], start=True, stop=True)
                    t1 = wpool.tile([WIDTH, SPC], F32, tag="t1")
                    nc.scalar.activation(t1[:], a1[:], mybir.ActivationFunctionType.Tanh, bias=bi1, scale=sc1)
                    sq1 = wpool.tile([WIDTH, SPC], F32, tag="sq1")
                    nc.scalar.activation(sq1[:], t1[:], mybir.ActivationFunctionType.Square)

                    # ---- layer 2 + gate -> k_i ----
                    fps = psF.tile([NUM_LATENTS, SPC], F32, tag="fps")
                    nc.tensor.matmul(fps[:], w2t[:], t1[:], start=True, stop=True)
                    kz_i = kz[:, i * SPC : (i + 1) * SPC]
                    nc.scalar.activation(kz_i, fps[:], mybir.ActivationFunctionType.Identity, bias=bi2, scale=sc2)

                    # ---- z accumulation: znext += dt*b_i * k_i ----
                    nc.vector.scalar_tensor_tensor(
                        nxt[0:32, :], kz_i, float(DT * TSIT_B[i]), nxt[0:32, :],
                        op0=mybir.AluOpType.mult, op1=mybir.AluOpType.add,
                    )

                    # ---- trace term: acc += sum_j (c - M')_j (1 - sq1)_j ----
                    mp = psM.tile([WIDTH, SPC], F32, tag="mp")
                    nc.tensor.matmul(mp[:], btc[:, i * WIDTH : (i + 1) * WIDTH], sq0[:], start=True, stop=True)
                    y1 = wpool.tile([WIDTH, SPC], F32, tag="y1")
                    nc.vector.scalar_tensor_tensor(
                        y1[:], mp[:], cvec, sq1[:],
                        op0=mybir.AluOpType.subtract, op1=mybir.AluOpType.mult,
                    )
                    zt = wpool.tile([WIDTH, SPC], F32, tag="zt")
                    nc.vector.scalar_tensor_tensor(
                        zt[:], mp[:], cvec, y1[:],
                        op0=mybir.AluOpType.subtract, op1=mybir.AluOpType.subtract,
                    )
                    nc.tensor.matmul(acc[:], negones, zt[:], start=(s == 0), stop=False, skip_group_check=True)
                    s += 1

            # ---- base log-prob of z_final under N(0, I) ----
            zf = (xzc0 if N_STEPS % 2 == 0 else xzc1)[0:32, :]
            sqz = wpool.tile([NUM_LATENTS, SPC], F32, tag="sqz")
            nc.scalar.activation(sqz[:], zf, mybir.ActivationFunctionType.Square)
            nc.tensor.matmul(acc[:], neghalf, sqz[:], start=False, stop=True, skip_group_check=True)

            fbias = sb32[0:1, NSTG * 2 + 1 : NSTG * 2 + 2]
            nc.scalar.activation(res[:], acc[:], mybir.ActivationFunctionType.Identity,
                                 bias=fbias)
            nc.sync.dma_start(out_d[:], res[:])

    nc.compile()
    return nc


def _host_constants(z, cond_vars, params):
    """Build all per-core input arrays (constants replicated, data sharded)."""
    params = [[np.asarray(p, dtype=np.float64) if p is not None else None for p in layer] for layer in params]
    (W1_0, b1_0, W2_0, b2_0, W3_0) = params[0]
    (W1_1, b1_1, W2_1, b2_1, W3_1) = params[1]
    (W1_2, b1_2, W2_2, b2_2, W3_2) = params[2]
    W0z = W1_0[:, :NUM_LATENTS]

    w0t = W1_0.T.astype(np.float32)
    w1t = W1_1.T.astype(np.float32)
    w2t = W1_2.T.astype(np.float32)

    wz = np.empty((NUM_LATENTS, N_APAIRS * WIDTH), dtype=np.float32)
    for (i, j), idx in _AIDX.items():
        wz[:, idx * WIDTH : (idx + 1) * WIDTH] = (DT * TSIT_A[i][j] * W0z).T

    bt = np.empty((WIDTH, NSTG * WIDTH), dtype=np.float32)
    sb128 = np.empty((WIDTH, NSTG * 5 + 1), dtype=np.float32)
    sb32 = np.empty((NUM_LATENTS, NSTG * 2 + 2), dtype=np.float32)
    s = 0
    for n in range(N_STEPS):
        t_n = T1 + DT * n
        for i in range(N_STAGES):
            t = t_n + TSIT_C[i] * DT
            g0 = 1.0 / (1.0 + np.exp(-(W2_0[:, 0] * t + b2_0)))
            g1 = 1.0 / (1.0 + np.exp(-(W2_1[:, 0] * t + b2_1)))
            g2 = 1.0 / (1.0 + np.exp(-(W2_2[:, 0] * t + b2_2)))
            sb128[:, 5 * s + 0] = g0
            sb128[:, 5 * s + 1] = b1_0 * g0 + W3_0[:, 0] * t
            sb128[:, 5 * s + 2] = g1
            sb128[:, 5 * s + 3] = b1_1 * g1 + W3_1[:, 0] * t
            sb32[:, 2 * s + 0] = g2
            sb32[:, 2 * s + 1] = b1_2 * g2 + W3_2[:, 0] * t
            k = DT * TSIT_B[i]
            P = W0z @ (g2[:, None] * W1_2)
            Btil = ((k * g0)[:, None] * (P * W1_1.T) * g1[None, :]).astype(np.float32)
            bt[:, s * WIDTH : (s + 1) * WIDTH] = Btil
            sb128[:, 5 * s + 4] = Btil.sum(axis=0, dtype=np.float64)
            s += 1
    sb128[:, NSTG * 5] = -1.0
    sb32[:, NSTG * 2] = -0.5
    sb32[:, NSTG * 2 + 1] = 0.0
    sb32[0, NSTG * 2 + 1] = -0.5 * NUM_LATENTS * LOG_2PI

    z = np.asarray(z, dtype=np.float32)
    cond_vars = np.asarray(cond_vars, dtype=np.float32)
    in_maps = []
    for c in range(N_CORES):
        zs = z[c * SPC : (c + 1) * SPC]          # (32, 32)
        cs = cond_vars[c * SPC : (c + 1) * SPC]  # (32, 32)
        xzc = np.concatenate([zs.T, cs.T], axis=0).astype(np.float32)  # (64, 32)
        in_maps.append({
            "xzc": xzc, "w0t": w0t, "w1t": w1t, "w2t": w2t,
            "wz": wz, "bt": bt, "sb128": sb128, "sb32": sb32,
        })
    return in_maps


_CACHE = {}


def _get_runner():
    """Compile the Bass program once and build a cached jitted executor."""
    if "runner" in _CACHE:
        return _CACHE["runner"]

    import jax
    from jax.sharding import Mesh, PartitionSpec
    from jax.experimental.shard_map import shard_map
    from concourse import bass2jax
    from concourse.bass2jax import _bass_exec_p, install_neuronx_cc_hook

    nc = _build_program()
    _CACHE["nc"] = nc
    install_neuronx_cc_hook()

    partition_name = nc.partition_id_tensor.name if nc.partition_id_tensor else None
    in_names, out_names, out_avals, zero_outs = [], [], [], []
    for alloc in nc.m.functions[0].allocations:
        if not isinstance(alloc, mybir.MemoryLocationSet):
            continue
        name = alloc.memorylocations[0].name
        if alloc.kind == "ExternalInput":
            if name != partition_name:
                in_names.append(name)
        elif alloc.kind == "ExternalOutput":
            out_names.append(name)
            shape = tuple(alloc.tensor_shape)
            dtype = mybir.dt.np(alloc.dtype)
            out_avals.append(jax.core.ShapedArray(shape, dtype))
            zero_outs.append(np.zeros(shape, dtype))
    n_params = len(in_names)
    all_in_names = in_names + out_names
    if partition_name is not None:
        all_in_names = all_in_names + [partition_name]

    def _body(*args):
        operands = list(args)
        if partition_name is not None:
            operands.append(bass2jax.partition_id_tensor())
        outs = _bass_exec_p.bind(
            *operands,
            out_avals=tuple(out_avals),
            in_names=tuple(all_in_names),
            out_names=tuple(out_names),
            lowering_input_output_aliases=(),
            sim_require_finite=True,
            sim_require_nnan=True,
            nc=nc,
        )
        return tuple(outs)

    devices = jax.devices()[:N_CORES]
    mesh = Mesh(np.asarray(devices), ("core",))
    in_specs = (PartitionSpec("core"),) * (n_params + len(out_names))
    out_specs = (PartitionSpec("core"),) * len(out_names)
    donate = tuple(range(n_params, n_params + len(out_names)))
    sharded = jax.jit(
        shard_map(_body, mesh=mesh, in_specs=in_specs, out_specs=out_specs, check_rep=False),
        donate_argnums=donate,
        keep_unused=True,
    )

    def run(in_maps):
        concat_in = [
            np.concatenate([np.asarray(in_maps[c][name]) for c in range(N_CORES)], axis=0)
            for name in in_names
        ]
        concat_zeros = [np.zeros((N_CORES * zo.shape[0], *zo.shape[1:]), zo.dtype) for zo in zero_outs]
        out_arrs = sharded(*concat_in, *concat_zeros)
        return [
            {name: np.asarray(out_arrs[k]).reshape(N_CORES, *out_avals[k].shape)[c]
             for k, name in enumerate(out_names)}
            for c in range(N_CORES)
        ]

    _CACHE["runner"] = run
    return run


def kernel(z, cond_vars, params):
    run = _get_runner()
    in_maps = _host_constants(z, cond_vars, params)
    results = run(in_maps)
    out = np.empty((BATCH,), dtype=np.float32)
    for c in range(N_CORES):
        out[c * SPC : (c + 1) * SPC] = results[c]["out"].reshape(SPC)
    return out
